# revision 6
# baseline (speedup 1.0000x reference)
"""Trainium2 Bass kernel for the GCM aspect-sentiment model.

Sharding: pure data parallelism — batch (32) split across 8 NeuronCores
(4 items/core); embedding table + all weights replicated.

Per-core plan (all matmuls bf16 with fp32 PSUM accumulation):
  - embedding rows gathered from DRAM via indirect DMA, cast bf16,
    PE-transposed to channel-major [D, B*L] padded layout
  - attention via 2nd-order expansion of tanh(cw+aw) in the small aspect
    term: score = U.ty - (V tx (1-tx^2)).ty^2 with U = V(1-tx^2); the
    l-constant term V.tx cancels in softmax.  This removes the
    [B,L1,L2,E] tanh entirely.
  - conv1/conv2 as 3-tap shifted matmuls; asp_w and asp_b folded into the
    aspect half of conv2 on the host.
  - highway + maxpool + classifier on-chip.
"""

import numpy as np
import ml_dtypes

import concourse.bacc as bacc
import concourse.mybir as mybir
import concourse.tile as tile
from concourse.bass import IndirectOffsetOnAxis
from concourse.masks import make_identity
from concourse.bass_utils import run_bass_kernel_spmd

B, L1, L2 = 32, 512, 16
D, C, NCLS = 300, 256, 3
K = 3
VOCAB = 50000
E = D + C
NCORES = 8
BL = B // NCORES          # batch per core
NL = BL * L1              # 2048 context tokens per core
NM = BL * L2              # 64 aspect tokens per core
LP = L1 + 4               # padded per-b stride (zero col at l=-1 and l=512)
MP = L2 + 2               # padded per-b aspect stride

bf16 = mybir.dt.bfloat16
f32 = mybir.dt.float32
i32 = mybir.dt.int32
AF = mybir.ActivationFunctionType
ALU = mybir.AluOpType
AX = mybir.AxisListType
np_bf16 = ml_dtypes.bfloat16

D_TILES = [(0, 128), (128, 128), (256, 44)]
C_TILES = [(0, 128), (128, 128)]
E_TILES = [(0, 128), (128, 128), (256, 128), (384, 128), (512, 44)]

_NC_CACHE = {}


def build_nc(stage=None):
    nc = bacc.Bacc("TRN2", target_bir_lowering=False, debug=False)

    # ---- DRAM I/O ----
    d_ctx_ids = nc.dram_tensor("ctx_ids", [NL, 1], i32, kind="ExternalInput")
    d_asp_ids = nc.dram_tensor("asp_ids", [NM, 1], i32, kind="ExternalInput")
    d_emb = nc.dram_tensor("wordemb", [VOCAB, D], f32, kind="ExternalInput")
    d_w1t = nc.dram_tensor("w1t", [D, E], bf16, kind="ExternalInput")
    d_w2t = nc.dram_tensor("w2t", [C, E], bf16, kind="ExternalInput")
    d_v2 = nc.dram_tensor("v2", [E, 2], f32, kind="ExternalInput")
    d_w3 = nc.dram_tensor("w3t", [D, K * C], bf16, kind="ExternalInput")
    d_w1c = nc.dram_tensor("w1ct", [D, K * C], bf16, kind="ExternalInput")
    d_w2ctx = nc.dram_tensor("w2ctxt", [D, K * C], bf16, kind="ExternalInput")
    d_w2att = nc.dram_tensor("w2attt", [C, K * C], bf16, kind="ExternalInput")
    d_hwt = nc.dram_tensor("hwt", [C, C], bf16, kind="ExternalInput")
    d_hwgt = nc.dram_tensor("hwgt", [C, C], bf16, kind="ExternalInput")
    d_outwt = nc.dram_tensor("outwt", [C, NCLS], bf16, kind="ExternalInput")
    d_bias = nc.dram_tensor("biases", [C, 5], f32, kind="ExternalInput")
    d_outb = nc.dram_tensor("outb", [BL, NCLS], f32, kind="ExternalInput")
    d_out = nc.dram_tensor("out", [BL, NCLS], f32, kind="ExternalOutput")

    with tile.TileContext(nc) as tc:
        _body(nc, tc, d_ctx_ids, d_asp_ids, d_emb, d_w1t, d_w2t, d_v2, d_w3,
              d_w1c, d_w2ctx, d_w2att, d_hwt, d_hwgt, d_outwt, d_bias, d_outb,
              d_out, stage=stage)
    nc.compile()
    return nc


def _body(nc, tc, d_ctx_ids, d_asp_ids, d_emb, d_w1t, d_w2t, d_v2, d_w3,
          d_w1c, d_w2ctx, d_w2att, d_hwt, d_hwgt, d_outwt, d_bias, d_outb,
          d_out, stage=None):
    import contextlib
    stack = contextlib.ExitStack()
    cst = stack.enter_context(tc.tile_pool(name="cst", bufs=1))
    per = stack.enter_context(tc.tile_pool(name="per", bufs=1))
    wk = stack.enter_context(tc.tile_pool(name="wk", bufs=3))
    ps = stack.enter_context(tc.tile_pool(name="ps", bufs=4, space="PSUM"))
    ps2 = stack.enter_context(tc.tile_pool(name="ps2", bufs=4, space="PSUM"))


    def finish(src):
        osb = wk.tile([BL, NCLS], f32, tag="osb", name="osb")
        nc.vector.tensor_copy(osb[:], src)
        nc.sync.dma_start(d_out.ap(), osb[:])
        stack.close()

    # ---- constants into SBUF ----
    ident = cst.tile([128, 128], bf16, tag="ident", name="ident")
    make_identity(nc, ident[:])

    w1t_sb = []
    for dt, (d0, dsz) in enumerate(D_TILES):
        t = cst.tile([dsz, E], bf16, tag=f"w1t{dt}", name=f"w1t{dt}")
        nc.sync.dma_start(t[:], d_w1t.ap()[d0:d0 + dsz, :])
        w1t_sb.append(t)
    w2t_sb = []
    for ct, (c0, csz) in enumerate(C_TILES):
        t = cst.tile([csz, E], bf16, tag=f"w2t{ct}", name=f"w2t{ct}")
        nc.sync.dma_start(t[:], d_w2t.ap()[c0:c0 + csz, :])
        w2t_sb.append(t)
    v2_sb = []
    for et, (e0, esz) in enumerate(E_TILES):
        t = cst.tile([esz, 2], f32, tag=f"v2{et}", name=f"v2{et}")
        nc.sync.dma_start(t[:], d_v2.ap()[e0:e0 + esz, :])
        v2_sb.append(t)

    def load_conv_w(dram, tiles, name):
        out = []
        for it, (o0, osz) in enumerate(tiles):
            t = cst.tile([osz, K * C], bf16, tag=f"{name}{it}", name=f"{name}{it}")
            nc.sync.dma_start(t[:], dram.ap()[o0:o0 + osz, :])
            out.append(t)
        return out

    w3_sb = load_conv_w(d_w3, D_TILES, "w3")
    w1c_sb = load_conv_w(d_w1c, D_TILES, "w1c")
    w2ctx_sb = load_conv_w(d_w2ctx, D_TILES, "w2ctx")
    w2att_sb = load_conv_w(d_w2att, C_TILES, "w2att")

    hwt_sb, hwgt_sb, outwt_sb, bias_sb = [], [], [], []
    for ct, (c0, csz) in enumerate(C_TILES):
        t = cst.tile([csz, C], bf16, tag=f"hwt{ct}", name=f"hwt{ct}")
        nc.sync.dma_start(t[:], d_hwt.ap()[c0:c0 + csz, :])
        hwt_sb.append(t)
        t = cst.tile([csz, C], bf16, tag=f"hwgt{ct}", name=f"hwgt{ct}")
        nc.sync.dma_start(t[:], d_hwgt.ap()[c0:c0 + csz, :])
        hwgt_sb.append(t)
        t = cst.tile([csz, NCLS], bf16, tag=f"outwt{ct}", name=f"outwt{ct}")
        nc.sync.dma_start(t[:], d_outwt.ap()[c0:c0 + csz, :])
        outwt_sb.append(t)
        t = cst.tile([csz, 5], f32, tag=f"bias{ct}", name=f"bias{ct}")
        nc.sync.dma_start(t[:], d_bias.ap()[c0:c0 + csz, :])
        bias_sb.append(t)
    outb_sb = cst.tile([BL, NCLS], f32, tag="outb", name="outb")
    nc.sync.dma_start(outb_sb[:], d_outb.ap())

    # ---- persistent activations ----
    ctxT = [per.tile([dsz, BL * LP], bf16, tag=f"ctxT{dt}", name=f"ctxT{dt}")
            for dt, (d0, dsz) in enumerate(D_TILES)]
    aspT = [per.tile([dsz, BL * MP], bf16, tag=f"aspT{dt}", name=f"aspT{dt}")
            for dt, (d0, dsz) in enumerate(D_TILES)]
    attT = [per.tile([csz, BL * LP], bf16, tag=f"attT{ct}", name=f"attT{ct}")
            for ct, (c0, csz) in enumerate(C_TILES)]
    for t in ctxT + aspT + attT:
        nc.gpsimd.memset(t[:], 0.0)
    UT = [per.tile([esz, NL], bf16, tag=f"UT{et}", name=f"UT{et}")
          for et, (e0, esz) in enumerate(E_TILES)]
    U2T = [per.tile([esz, NL], bf16, tag=f"U2T{et}", name=f"U2T{et}")
           for et, (e0, esz) in enumerate(E_TILES)]
    tyT = [per.tile([esz, NM], bf16, tag=f"tyT{et}", name=f"tyT{et}")
           for et, (e0, esz) in enumerate(E_TILES)]
    nty2T = [per.tile([esz, NM], bf16, tag=f"nty2T{et}", name=f"nty2T{et}")
             for et, (e0, esz) in enumerate(E_TILES)]
    aT = [per.tile([csz, NM], bf16, tag=f"aT{ct}", name=f"aT{ct}")
          for ct, (c0, csz) in enumerate(C_TILES)]
    a_b = [per.tile([L2, C], bf16, tag=f"a_b{b}", name=f"a_b{b}") for b in range(BL)]
    alphaT = per.tile([L2, NL], bf16, tag="alphaT", name="alphaT")
    mT = [per.tile([csz, NL], bf16, tag=f"mT{ct}", name=f"mT{ct}")
          for ct, (c0, csz) in enumerate(C_TILES)]
    pooled = [per.tile([csz, BL], bf16, tag=f"pooled{ct}", name=f"pooled{ct}")
              for ct, (c0, csz) in enumerate(C_TILES)]

    if stage == 0:
        return finish(ident[0:BL, 0:NCLS])

    # ---- context gather + transpose ----
    for t in range(NL // 128):
        b, lc = t // 4, t % 4
        idx = wk.tile([128, 1], i32, tag="idx", name="idx")
        nc.sync.dma_start(idx[:], d_ctx_ids.ap()[t * 128:(t + 1) * 128, :])
        g = wk.tile([128, D], f32, tag="gath", name="gath")
        nc.gpsimd.indirect_dma_start(
            out=g[:], out_offset=None, in_=d_emb.ap(),
            in_offset=IndirectOffsetOnAxis(ap=idx[:, 0:1], axis=0))
        gb = wk.tile([128, D], bf16, tag="gathb", name="gathb")
        nc.vector.tensor_copy(gb[:], g[:])
        for dt, (d0, dsz) in enumerate(D_TILES):
            tr = ps2.tile([128, 128], bf16, tag="sm", name="tr")
            nc.tensor.transpose(out=tr[:dsz, :], in_=gb[:, d0:d0 + dsz],
                                identity=ident[:])
            col = b * LP + 1 + lc * 128
            nc.vector.tensor_copy(ctxT[dt][:, col:col + 128], tr[:dsz, :])

    if stage == 1:
        return finish(ctxT[0][0:BL, 0:NCLS])

    # ---- aspect branch ----
    idxa = wk.tile([NM, 1], i32, tag="idxa", name="idxa")
    nc.sync.dma_start(idxa[:], d_asp_ids.ap())
    ga = wk.tile([NM, D], f32, tag="gatha", name="gatha")
    nc.gpsimd.indirect_dma_start(
        out=ga[:], out_offset=None, in_=d_emb.ap(),
        in_offset=IndirectOffsetOnAxis(ap=idxa[:, 0:1], axis=0))
    gab = wk.tile([NM, D], bf16, tag="gathab", name="gathab")
    nc.vector.tensor_copy(gab[:], ga[:])
    for dt, (d0, dsz) in enumerate(D_TILES):
        tr = ps2.tile([128, 128], bf16, tag="sm", name="tr")
        nc.tensor.transpose(out=tr[:dsz, :NM], in_=gab[:, d0:d0 + dsz],
                            identity=ident[:NM, :NM])
        for b in range(BL):
            nc.vector.tensor_copy(
                aspT[dt][:, b * MP + 1:b * MP + 1 + L2],
                tr[:dsz, b * L2:(b + 1) * L2])

    # conv3 + relu -> aT  (out view [c, b, m])
    for ct, (c0, csz) in enumerate(C_TILES):
        pa = ps2.tile([128, NM], f32, tag="sm", name="pa")
        pa_v = pa[:csz, :].rearrange("p (b m) -> p b m", m=L2)
        first = True
        for k in range(K):
            for dt, (d0, dsz) in enumerate(D_TILES):
                rhs = aspT[dt][:].rearrange("p (b w) -> p b w", w=MP)[:, :, k:k + L2]
                nc.tensor.matmul(pa_v, w3_sb[dt][:, k * C + c0:k * C + c0 + csz],
                                 rhs, start=first,
                                 stop=(k == K - 1 and dt == len(D_TILES) - 1))
                first = False
        nc.scalar.activation(aT[ct][:], pa[:csz, :], AF.Relu,
                             bias=bias_sb[ct][:, 0:1])
    # a_b: per-batch [m, c] copies via transpose
    for b in range(BL):
        for ct, (c0, csz) in enumerate(C_TILES):
            tr = ps2.tile([128, 128], bf16, tag="sm", name="tr")
            nc.tensor.transpose(out=tr[:L2, :csz],
                                in_=aT[ct][:, b * L2:(b + 1) * L2],
                                identity=ident[:csz, :csz])
            nc.vector.tensor_copy(a_b[b][:, c0:c0 + csz], tr[:L2, :csz])
    # aw -> ty, -ty^2
    for et, (e0, esz) in enumerate(E_TILES):
        paw = ps2.tile([128, NM], f32, tag="sm", name="paw")
        for ct, (c0, csz) in enumerate(C_TILES):
            nc.tensor.matmul(paw[:esz, :], w2t_sb[ct][:, e0:e0 + esz], aT[ct][:],
                             start=(ct == 0), stop=(ct == len(C_TILES) - 1))
        nc.scalar.activation(tyT[et][:], paw[:esz, :], AF.Tanh)
        ty2 = wk.tile([128, NM], bf16, tag="ty2", name="ty2")
        nc.vector.tensor_tensor(ty2[:esz, :], tyT[et][:], tyT[et][:], op=ALU.mult)
        nc.vector.tensor_scalar_mul(nty2T[et][:], ty2[:esz, :], -1.0)

    if stage == 2:
        return finish(tyT[0][0:BL, 0:NCLS])

    # ---- cw -> tx -> U, U2 ----
    for b in range(BL):
        for et, (e0, esz) in enumerate(E_TILES):
            pcw = ps.tile([128, L1], f32, tag="mm", name="mm")
            for dt, (d0, dsz) in enumerate(D_TILES):
                nc.tensor.matmul(pcw[:esz, :], w1t_sb[dt][:, e0:e0 + esz],
                                 ctxT[dt][:, b * LP + 1:b * LP + 1 + L1],
                                 start=(dt == 0), stop=(dt == len(D_TILES) - 1))
            tx = wk.tile([128, L1], bf16, tag="tx", name="tx")
            nc.scalar.activation(tx[:esz, :], pcw[:esz, :], AF.Tanh)
            sq = wk.tile([128, L1], bf16, tag="sq", name="sq")
            nc.vector.tensor_tensor(sq[:esz, :], tx[:esz, :], tx[:esz, :],
                                    op=ALU.mult)
            us = UT[et][:, b * L1:(b + 1) * L1]
            nc.vector.tensor_scalar(us, sq[:esz, :],
                                    v2_sb[et][:, 1:2], v2_sb[et][:, 0:1],
                                    op0=ALU.mult, op1=ALU.add)
            nc.vector.tensor_tensor(U2T[et][:, b * L1:(b + 1) * L1],
                                    tx[:esz, :], us, op=ALU.mult)

    if stage == 3:
        return finish(UT[0][0:BL, 0:NCLS])

    # ---- score -> softmax -> alphaT ----
    for b in range(BL):
        for lc in range(L1 // 128):
            psc = ps2.tile([128, L2], f32, tag="sm", name="sc")
            col = b * L1 + lc * 128
            n_et = len(E_TILES)
            for et, (e0, esz) in enumerate(E_TILES):
                nc.tensor.matmul(psc[:], UT[et][:esz, col:col + 128],
                                 tyT[et][:, b * L2:(b + 1) * L2],
                                 start=(et == 0), stop=False)
            for et, (e0, esz) in enumerate(E_TILES):
                nc.tensor.matmul(psc[:], U2T[et][:esz, col:col + 128],
                                 nty2T[et][:, b * L2:(b + 1) * L2],
                                 start=False, stop=(et == n_et - 1))
            al_u = wk.tile([128, L2], bf16, tag="alu", name="alu")
            rs = wk.tile([128, 1], f32, tag="rs", name="rs")
            nc.scalar.activation(al_u[:], psc[:], AF.Exp, accum_out=rs[:])
            rc = wk.tile([128, 1], f32, tag="rc", name="rc")
            nc.vector.reciprocal(rc[:], rs[:])
            al = wk.tile([128, L2], bf16, tag="al", name="al")
            nc.vector.tensor_scalar_mul(al[:], al_u[:], rc[:, 0:1])
            tr = ps2.tile([128, 128], bf16, tag="sm", name="tr")
            nc.tensor.transpose(out=tr[:L2, :], in_=al[:], identity=ident[:])
            nc.vector.tensor_copy(alphaT[:, col:col + 128], tr[:L2, :])

    if stage == 4:
        return finish(alphaT[0:BL, 0:NCLS])

    # ---- att (normalized) ----
    for b in range(BL):
        for ct, (c0, csz) in enumerate(C_TILES):
            pat = ps.tile([128, L1], f32, tag="mm", name="mm")
            nc.tensor.matmul(pat[:csz, :], a_b[b][:, c0:c0 + csz],
                             alphaT[:, b * L1:(b + 1) * L1],
                             start=True, stop=True)
            nc.vector.tensor_copy(
                attT[ct][:, b * LP + 1:b * LP + 1 + L1], pat[:csz, :])

    # ---- conv1 (tanh) and conv2 (relu, asp folded) -> m ----
    for b in range(BL):
        for ct, (c0, csz) in enumerate(C_TILES):
            ps1 = ps.tile([128, L1], f32, tag="mm", name="mm")
            first = True
            for k in range(K):
                for dt, (d0, dsz) in enumerate(D_TILES):
                    nc.tensor.matmul(
                        ps1[:csz, :], w1c_sb[dt][:, k * C + c0:k * C + c0 + csz],
                        ctxT[dt][:, b * LP + k:b * LP + k + L1],
                        start=first, stop=(k == K - 1 and dt == len(D_TILES) - 1))
                    first = False
            pg = ps.tile([128, L1], f32, tag="mm", name="mm")
            first = True
            for k in range(K):
                for dt, (d0, dsz) in enumerate(D_TILES):
                    nc.tensor.matmul(
                        pg[:csz, :], w2ctx_sb[dt][:, k * C + c0:k * C + c0 + csz],
                        ctxT[dt][:, b * LP + k:b * LP + k + L1],
                        start=first, stop=False)
                    first = False
            for k in range(K):
                for jt, (j0, jsz) in enumerate(C_TILES):
                    nc.tensor.matmul(
                        pg[:csz, :], w2att_sb[jt][:, k * C + c0:k * C + c0 + csz],
                        attT[jt][:, b * LP + k:b * LP + k + L1],
                        start=False, stop=(k == K - 1 and jt == len(C_TILES) - 1))
            s1 = wk.tile([128, L1], bf16, tag="s1", name="s1")
            nc.scalar.activation(s1[:csz, :], ps1[:csz, :], AF.Tanh,
                                 bias=bias_sb[ct][:, 1:2])
            gg = wk.tile([128, L1], bf16, tag="gg", name="gg")
            nc.scalar.activation(gg[:csz, :], pg[:csz, :], AF.Relu,
                                 bias=bias_sb[ct][:, 2:3])
            nc.vector.tensor_tensor(mT[ct][:, b * L1:(b + 1) * L1],
                                    s1[:csz, :], gg[:csz, :], op=ALU.mult)

    if stage == 5:
        return finish(mT[0][0:BL, 0:NCLS])

    # ---- highway + maxpool ----
    for b in range(BL):
        for ct, (c0, csz) in enumerate(C_TILES):
            ph = ps.tile([128, L1], f32, tag="mm", name="mm")
            phg = ps.tile([128, L1], f32, tag="mm", name="mm")
            for jt, (j0, jsz) in enumerate(C_TILES):
                nc.tensor.matmul(ph[:csz, :], hwt_sb[jt][:, c0:c0 + csz],
                                 mT[jt][:, b * L1:(b + 1) * L1],
                                 start=(jt == 0), stop=(jt == len(C_TILES) - 1))
            for jt, (j0, jsz) in enumerate(C_TILES):
                nc.tensor.matmul(phg[:csz, :], hwgt_sb[jt][:, c0:c0 + csz],
                                 mT[jt][:, b * L1:(b + 1) * L1],
                                 start=(jt == 0), stop=(jt == len(C_TILES) - 1))
            hh = wk.tile([128, L1], bf16, tag="hh", name="hh")
            nc.scalar.activation(hh[:csz, :], ph[:csz, :], AF.Relu,
                                 bias=bias_sb[ct][:, 3:4])
            gt = wk.tile([128, L1], bf16, tag="gt", name="gt")
            nc.scalar.activation(gt[:csz, :], phg[:csz, :], AF.Sigmoid,
                                 bias=bias_sb[ct][:, 4:5])
            dd = wk.tile([128, L1], bf16, tag="dd", name="dd")
            nc.vector.tensor_tensor(dd[:csz, :], hh[:csz, :],
                                    mT[ct][:, b * L1:(b + 1) * L1], op=ALU.subtract)
            ee = wk.tile([128, L1], bf16, tag="ee", name="ee")
            nc.vector.tensor_tensor(ee[:csz, :], gt[:csz, :], dd[:csz, :],
                                    op=ALU.mult)
            m2 = wk.tile([128, L1], bf16, tag="m2", name="m2")
            nc.vector.tensor_tensor(m2[:csz, :], ee[:csz, :],
                                    mT[ct][:, b * L1:(b + 1) * L1], op=ALU.add)
            nc.vector.reduce_max(out=pooled[ct][:, b:b + 1], in_=m2[:csz, :],
                                 axis=AX.X)

    # ---- classifier ----
    po = ps2.tile([128, L2], f32, tag="sm", name="po")
    for ct, (c0, csz) in enumerate(C_TILES):
        nc.tensor.matmul(po[:BL, :NCLS], pooled[ct][:], outwt_sb[ct][:],
                         start=(ct == 0), stop=(ct == len(C_TILES) - 1))
    osb = wk.tile([BL, NCLS], f32, tag="osb", name="osb")
    nc.vector.tensor_tensor(osb[:], po[:BL, :NCLS], outb_sb[:], op=ALU.add)
    nc.sync.dma_start(d_out.ap(), osb[:])
    stack.close()


def prep_inputs(context_ids, aspect_ids, wordemb, conv3_w, conv3_b, conv1_w,
                conv1_b, conv2_w, conv2_b, attn_W, attn_V, asp_w, asp_b, hw_w,
                hw_b, hwg_w, hwg_b, out_w, out_b):
    """Host-side prep: weight layout transforms + bf16 casts (weights only)."""
    f = np.float32
    attn_W = np.asarray(attn_W, f)
    w2 = np.asarray(conv2_w, f)
    asp_w = np.asarray(asp_w, f)

    shared = {
        "wordemb": np.asarray(wordemb, f),
        "w1t": np.ascontiguousarray(attn_W[:, :D].T).astype(np_bf16),
        "w2t": np.ascontiguousarray(attn_W[:, D:].T).astype(np_bf16),
        "v2": np.stack([np.asarray(attn_V, f)[0], -np.asarray(attn_V, f)[0]],
                       axis=1).astype(f),
        "w3t": np.asarray(conv3_w, f).transpose(1, 2, 0).reshape(D, K * C)
              .astype(np_bf16),
        "w1ct": np.asarray(conv1_w, f).transpose(1, 2, 0).reshape(D, K * C)
               .astype(np_bf16),
        "w2ctxt": w2[:, :D, :].transpose(1, 2, 0).reshape(D, K * C)
                 .astype(np_bf16),
        "w2attt": np.einsum("aok,oc->ack", w2[:, D:, :], asp_w)
                 .transpose(1, 2, 0).reshape(C, K * C).astype(np_bf16),
        "hwt": np.ascontiguousarray(np.asarray(hw_w, f).T).astype(np_bf16),
        "hwgt": np.ascontiguousarray(np.asarray(hwg_w, f).T).astype(np_bf16),
        "outwt": np.ascontiguousarray(np.asarray(out_w, f).T).astype(np_bf16),
        "biases": np.stack([
            np.asarray(conv3_b, f),
            np.asarray(conv1_b, f),
            np.asarray(conv2_b, f) + np.einsum("aok,o->a", w2[:, D:, :],
                                               np.asarray(asp_b, f)),
            np.asarray(hw_b, f),
            np.asarray(hwg_b, f)], axis=1).astype(f),
        "outb": np.tile(np.asarray(out_b, f).reshape(1, NCLS), (BL, 1)),
    }
    in_maps = []
    for c in range(NCORES):
        m = dict(shared)
        m["ctx_ids"] = np.ascontiguousarray(
            np.asarray(context_ids, np.int32)[c * BL:(c + 1) * BL]
        ).reshape(NL, 1)
        m["asp_ids"] = np.ascontiguousarray(
            np.asarray(aspect_ids, np.int32)[c * BL:(c + 1) * BL]
        ).reshape(NM, 1)
        in_maps.append(m)
    return in_maps


def kernel(**inputs):
    if "nc" not in _NC_CACHE:
        _NC_CACHE["nc"] = build_nc()
    nc = _NC_CACHE["nc"]
    in_maps = prep_inputs(**inputs)
    res = run_bass_kernel_spmd(nc, in_maps, core_ids=list(range(NCORES)))
    return np.concatenate([res.results[c]["out"] for c in range(NCORES)], axis=0)


if __name__ == "__main__":
    rng = np.random.default_rng(0)
    print("building...")
    nc = build_nc()
    print("built ok")
